# revision 24
# baseline (speedup 1.0000x reference)
"""Trainium2 Bass kernel for nn_NodeAttentionPerMetaPath (GAT-style node attention).

Reference computation (N=8192, F_IN=256, d=64):
    h      = x @ trans                      # [N, d]
    e1     = h @ attn[:d];  e2 = h @ attn[d:]
    scores = leaky_relu(e1 + e2.T, 0.2)     # [N, N]
    masked = where(mask==0, -1e15, scores)
    out    = softmax(masked, axis=1) @ h    # [N, d]

Sharding: rows r across 8 cores (1024 each); every core computes the full
h/e locally from a replicated fp16 x (no collectives at all).

Algebra (exp monotone, alpha<1):
    exp(leaky(e1+e2)) / exp(a*e1) = B2[j] * max(C[r]*D[j], 1)
    C = exp(.8 e1), D = exp(.8 e2), B2 = exp(.2 e2), C*D*B2 = C*exp(e2)
    out = (P @ h) / (P @ 1),  P = mask * B2 * max(CD, 1)
        = mask * max(C_rep * db, b2)        (one fused 4x tensor_scalar)

Layout: scores live TRANSPOSED [j-part, r-free] from the start.  The mask
is packed to fp16 on the host and transpose-loaded straight from DRAM by
the DMA XBAR (dma_start(transpose=True)), so the [N,N] work needs NO PE
transposes and NO PSUM->SBUF copies: per 128-j chunk it is one TS, one
masked multiply (split DVE/GPSIMD), and one accumulated matmul whose
ones-column yields the softmax denominator for free.
"""

from contextlib import ExitStack

import numpy as np

import concourse.bass as bass
import concourse.bacc as bacc
import concourse.mybir as mybir
import concourse.tile as tile
from concourse.bass_utils import run_bass_kernel_spmd
from concourse.masks import make_identity

f32 = mybir.dt.float32
fp16 = mybir.dt.float16

Exp = mybir.ActivationFunctionType.Exp

N_CORES = 8
N = 8192
F_IN = 256
D = 64
ALPHA = 0.2

R = N // N_CORES   # rows per core
JC = N // 128      # j-chunks
PC = JC // 2       # packed chunks (mask pairs 2 j-chunks per fp16)
NG = 8             # mask xbar groups
PG = PC // NG      # packed chunks per group (4)
HQ = 16            # h-compute groups (4 chunks each)

# masked-multiply split per chunk-PAIR [128, 2048]:
# columns [0:TT_SPLIT] on DVE, rest on GPSIMD
TT_SPLIT = 672


def build_kernel(ctx: ExitStack, tc: tile.TileContext, mask16_c, x_h, trans_h, transT_h, a12h, outT):
    nc = tc.nc

    singles = ctx.enter_context(tc.tile_pool(name="singles", bufs=1))
    hps_pool = ctx.enter_context(tc.tile_pool(name="hps", bufs=2, space="PSUM"))
    ps_o = ctx.enter_context(tc.tile_pool(name="ps_o", bufs=1, space="PSUM"))
    ps_r = ctx.enter_context(tc.tile_pool(name="ps_r", bufs=1, space="PSUM"))
    work = ctx.enter_context(tc.tile_pool(name="work", bufs=3))
    outp = ctx.enter_context(tc.tile_pool(name="outp", bufs=2))
    dram = ctx.enter_context(tc.tile_pool(name="dram", bufs=1, space="DRAM"))

    # ---- DMA streams.  ALL XBAR transposes share one HWDGE queue (sync):
    # concurrent XBAR streams on two queues corrupt each other.
    # x first (it unblocks all compute), then the mask.
    trans_w = singles.tile([128, 2, D + 2], fp16)
    nc.scalar.dma_start(
        out=trans_w[:, :, 0:D], in_=trans_h.rearrange("(c p) d -> p c d", p=128)
    )
    transT_sb = singles.tile([D, F_IN], fp16)
    nc.scalar.dma_start(out=transT_sb, in_=transT_h[:, :])
    a12_sb = singles.tile([D, 2], fp16)
    nc.scalar.dma_start(out=a12_sb, in_=a12h[:, :])
    # xT[p, fc, j] = x[j, fc*128+p] in one XBAR
    xT = singles.tile([128, 2, N], fp16)
    nc.sync.dma_start(out=xT, in_=x_h[:, :], transpose=True)

    # packed-mask transpose-stream: host packs chunk pairs into one fp16:
    # packed[r, c'*128+p] = m[r, 2c'*128+p] + 512*m[r, (2c'+1)*128+p].
    # mt_g[p, k, r] = packed[r, (g*PG+k)*128+p]
    # rotating buffer pool: the XBAR stream self-throttles on consumption
    mtp = ctx.enter_context(tc.tile_pool(name="mtp", bufs=5))
    mts = []
    for g in range(NG):
        mt = mtp.tile([128, PG, R], fp16, tag="mt", name=f"mt{g}")
        cols = slice(g * PG * 128, (g + 1) * PG * 128)
        nc.sync.dma_start(out=mt, in_=mask16_c[:, cols], transpose=True)
        mts.append(mt)

    ident = singles.tile([128, 128], fp16)
    make_identity(nc, ident)
    ones_row_f = singles.tile([1, D], f32)
    nc.vector.memset(ones_row_f, 1.0)

    # ---- w12 = trans.T @ a12  -> moving-operand columns 64:66
    for fc in range(2):
        wps = hps_pool.tile([128, 2], f32, tag="wps", bufs=1)
        nc.tensor.matmul(
            wps, transT_sb[:, fc * 128 : (fc + 1) * 128], a12_sb, start=True, stop=True
        )
        nc.vector.tensor_copy(trans_w[:, fc, D : D + 2], wps)

    # ---- full h/e per 4-chunk groups.  Inputs are rolled per core so the
    # own 1024 rows are always chunks 0..7 (groups 0 and 1).
    h_sb = singles.tile([128, JC, D + 1], fp16)   # [j%128, jc, h | 1]
    nc.vector.memset(h_sb[:, :, D], 1.0)
    nln512 = singles.tile([128, 1], f32)
    nc.vector.memset(nln512, -float(np.log(512.0)))
    c_all = singles.tile([128, JC], fp16)   # exp(.8 e1)
    db_all = singles.tile([128, JC], f32)  # exp(e2)
    b2_all = singles.tile([128, JC], f32)  # exp(.2 e2)
    C_rep = singles.tile([128, R], fp16)

    for q in range(HQ):
        hps = hps_pool.tile([128, 4, D + 2], f32, tag="hps")
        for k in range(4):
            jc = q * 4 + k
            j0 = jc * 128
            for fc in range(2):
                nc.tensor.matmul(
                    hps[:, k, :],
                    xT[:, fc, j0 : j0 + 128],
                    trans_w[:, fc, :],
                    start=(fc == 0),
                    stop=(fc == 1),
                )
        cols = slice(q * 4, q * 4 + 4)
        nc.scalar.copy(h_sb[:, cols, 0:D], hps[:, :, 0:D])
        nc.scalar.activation(c_all[:, cols], hps[:, :, D], Exp, scale=1.0 - ALPHA)
        # odd j-chunks carry the packed mask's high bit as {0, 512}; divide
        # their exponentials by 512 (bias -ln 512) to compensate exactly
        ce = slice(q * 4, q * 4 + 4, 2)
        co = slice(q * 4 + 1, q * 4 + 4, 2)
        nc.scalar.activation(db_all[:, ce], hps[:, 0::2, D + 1], Exp, scale=1.0)
        nc.scalar.activation(b2_all[:, ce], hps[:, 0::2, D + 1], Exp, scale=ALPHA)
        nc.scalar.activation(db_all[:, co], hps[:, 1::2, D + 1], Exp, scale=1.0, bias=nln512)
        nc.scalar.activation(b2_all[:, co], hps[:, 1::2, D + 1], Exp, scale=ALPHA, bias=nln512)

        if q == 1:
            # ---- C_rep: own-row C values replicated across partitions.
            # Transpose each own column of c_all to a [1, 128] row (PE),
            # assemble [1, R], then broadcast via K=1 matmuls.
            # (No DRAM roundtrip, no DMA in the middle of the XBAR stream.)
            c_row1 = singles.tile([1, R], fp16)
            crps = hps_pool.tile([1, 8, 128], fp16, tag="crps", bufs=1)
            for rb in range(8):
                nc.tensor.transpose(crps[:, rb, :], c_all[:, rb : rb + 1], ident)
            nc.vector.tensor_copy(c_row1, crps.rearrange("p a b -> p (a b)"))
            ones_col = singles.tile([1, 128], fp16)
            nc.vector.memset(ones_col, 1.0)
            for half in range(2):
                crp = hps_pool.tile([128, 512], f32, tag="crp", bufs=1)
                nc.tensor.matmul(
                    crp,
                    ones_col,
                    c_row1[:, half * 512 : (half + 1) * 512],
                    start=True,
                    stop=True,
                )
                nc.vector.tensor_copy(C_rep[:, half * 512 : (half + 1) * 512], crp)

    # ---- main loop: chunk-PAIRS.  Two 4x TS fills, one DVE TT + one
    # GPSIMD TT over the flattened [128, 2048], four accumulated matmuls.
    po = [ps_o.tile([D + 1, 512], f32, tag=f"po{i}", name=f"po{i}") for i in range(2)]
    M = mybir.AluOpType.mult
    # software-pipelined: the mask extraction for pair p+1 is emitted on the
    # DVE queue BEFORE pair p's TT so GPSIMD always has a pair of lead time.
    NP = JC // 2
    m2s = {}

    def extract(p):
        # unpack: m_high512 = (packed >= 512)*512 (exact), m_low = packed -
        # m_high512 (exact).  The 512 scale on odd chunks cancels against
        # the -ln512 bias folded into their db/b2 exponentials.
        mp = mts[p // PG][:, p % PG, :]
        m2 = work.tile([128, 2, R], fp16, tag="m2", bufs=8, name=f"m2_{p}")
        nc.vector.tensor_scalar(
            m2[:, 1, :], mp, 512.0, 512.0, mybir.AluOpType.is_ge, M
        )
        nc.vector.tensor_tensor(m2[:, 0, :], mp, m2[:, 1, :], mybir.AluOpType.subtract)
        m2s[p] = m2

    extract(0)
    extract(1)
    for p in range(NP):
        jc0 = 2 * p
        m2 = m2s.pop(p)
        vp = work.tile([128, 2, R], fp16, tag="vp", bufs=8)
        for half in range(2):
            jc = jc0 + half
            # v = max(C_rep * exp(e2[jc]), exp(.2 e2[jc]))  (= B2 * max(CD, 1))
            nc.vector.tensor_scalar(
                vp[:, half, :],
                C_rep,
                db_all[:, jc : jc + 1],
                b2_all[:, jc : jc + 1],
                M,
                mybir.AluOpType.max,
            )
        # P = mask * v -> pf, split DVE / GPSIMD; GPSIMD piece first (it has
        # everything it needs and is the long pole)
        pf = work.tile([128, 2, R], fp16, tag="pf", bufs=8)
        vf = vp.rearrange("p a b -> p (a b)")
        pff = pf.rearrange("p a b -> p (a b)")
        mf = m2.rearrange("p a b -> p (a b)")
        nc.gpsimd.tensor_tensor(pff[:, TT_SPLIT:], vf[:, TT_SPLIT:], mf[:, TT_SPLIT:], M)
        if p + 2 < NP:
            extract(p + 2)
        nc.vector.tensor_tensor(pff[:, 0:TT_SPLIT], vf[:, 0:TT_SPLIT], mf[:, 0:TT_SPLIT], M)
        for half in range(2):
            jc = jc0 + half
            for i in range(2):
                nc.tensor.matmul(
                    po[i],
                    h_sb[:, jc, :],
                    pf[:, half, i * 512 : (i + 1) * 512],
                    start=(jc == 0),
                    stop=(jc == JC - 1),
                )

    # ---- normalize: out = numer * (1/denom).  Broadcast the denominator
    # row via a K=1 outer product FIRST, then reciprocal on 64 lanes.
    for i in range(2):
        dcp = outp.tile([1, 512], f32, tag="dcp")
        nc.vector.tensor_copy(dcp, po[i][D : D + 1, :])
        rr = ps_r.tile([D, 512], f32, tag="rr")
        nc.tensor.matmul(rr, ones_row_f, dcp, start=True, stop=True)
        rr_sb = outp.tile([D, 512], f32, tag="rr_sb")
        nc.vector.reciprocal(rr_sb, rr)
        o_t = outp.tile([D, 512], f32, tag="o_t")
        nc.vector.tensor_tensor(o_t, po[i][0:D, :], rr_sb, M)
        nc.gpsimd.dma_start(out=outT[:, i * 512 : (i + 1) * 512], in_=o_t)


def build_nc():
    nc = bacc.Bacc("TRN2", num_devices=N_CORES)
    mask16_c = nc.dram_tensor("mask16_c", [R, N // 2], fp16, kind="ExternalInput")
    x_h = nc.dram_tensor("x_h", [N, F_IN], fp16, kind="ExternalInput")
    trans_h = nc.dram_tensor("trans_h", [F_IN, D], fp16, kind="ExternalInput")
    transT_h = nc.dram_tensor("transT_h", [D, F_IN], fp16, kind="ExternalInput")
    a12h = nc.dram_tensor("a12h", [D, 2], fp16, kind="ExternalInput")
    outT = nc.dram_tensor("outT", [D, R], f32, kind="ExternalOutput")
    with ExitStack() as ctx:
        tc = ctx.enter_context(tile.TileContext(nc))
        build_kernel(
            ctx, tc, mask16_c[:, :], x_h[:, :], trans_h[:, :],
            transT_h[:, :], a12h[:, :], outT[:, :],
        )
    nc.compile()
    return nc


LAST_RESULTS = None


def kernel(x, mask, trans, attn, _trace=False):
    x16 = np.ascontiguousarray(np.asarray(x), dtype=np.float16)
    mask16 = np.ascontiguousarray(np.asarray(mask), dtype=np.float16)
    trans16 = np.ascontiguousarray(np.asarray(trans), dtype=np.float16)
    transT16 = np.ascontiguousarray(np.asarray(trans).T, dtype=np.float16)
    attn = np.asarray(attn, dtype=np.float16)
    a12 = np.ascontiguousarray(np.concatenate([attn[:D], attn[D:]], axis=1))

    nc = build_nc()
    # identical SPMD program on every core: roll x rows / mask columns by
    # -c*R so each core's own rows are always j-chunks 0..7 (a column
    # permutation inside the softmax sum; the result is unchanged)
    def pack(mrows, c):
        mr = np.roll(mrows, -c * R, axis=1).reshape(R, JC // 2, 2, 128)
        return np.ascontiguousarray(mr[:, :, 0, :] + 512.0 * mr[:, :, 1, :], dtype=np.float16)

    in_maps = [
        {
            "mask16_c": pack(mask16[c * R : (c + 1) * R], c),
            "x_h": np.ascontiguousarray(np.roll(x16, -c * R, axis=0)),
            "trans_h": trans16,
            "transT_h": transT16,
            "a12h": a12,
        }
        for c in range(N_CORES)
    ]
    res = run_bass_kernel_spmd(nc, in_maps, list(range(N_CORES)), trace=_trace)
    global LAST_RESULTS
    LAST_RESULTS = res
    out = np.concatenate(
        [res.results[c]["outT"].T for c in range(N_CORES)], axis=0
    )
    return np.ascontiguousarray(out, dtype=np.float32)


if __name__ == "__main__":
    nc = build_nc()
    print("built OK")


# revision 25
# speedup vs baseline: 1.2696x; 1.2696x over previous
"""Trainium2 Bass kernel for nn_NodeAttentionPerMetaPath (GAT-style node attention).

Reference computation (N=8192, F_IN=256, d=64):
    h      = x @ trans                      # [N, d]
    e1     = h @ attn[:d];  e2 = h @ attn[d:]
    scores = leaky_relu(e1 + e2.T, 0.2)     # [N, N]
    masked = where(mask==0, -1e15, scores)
    out    = softmax(masked, axis=1) @ h    # [N, d]

Sharding: rows r across 8 cores (1024 each); every core computes the full
h/e locally from a replicated fp16 x (no collectives at all).

Algebra (exp monotone, alpha<1):
    exp(leaky(e1+e2)) / exp(a*e1) = B2[j] * max(C[r]*D[j], 1)
    C = exp(.8 e1), D = exp(.8 e2), B2 = exp(.2 e2), C*D*B2 = C*exp(e2)
    out = (P @ h) / (P @ 1),  P = mask * B2 * max(CD, 1)
        = mask * max(C_rep * db, b2)        (one fused 4x tensor_scalar)

Layout: scores live TRANSPOSED [j-part, r-free] from the start.  The mask
is packed to fp16 on the host and transpose-loaded straight from DRAM by
the DMA XBAR (dma_start(transpose=True)), so the [N,N] work needs NO PE
transposes and NO PSUM->SBUF copies: per 128-j chunk it is one TS, one
masked multiply (split DVE/GPSIMD), and one accumulated matmul whose
ones-column yields the softmax denominator for free.
"""

from contextlib import ExitStack

import numpy as np

import concourse.bass as bass
import concourse.bacc as bacc
import concourse.mybir as mybir
import concourse.tile as tile
from concourse.bass_utils import run_bass_kernel_spmd
from concourse.masks import make_identity

f32 = mybir.dt.float32
fp16 = mybir.dt.float16

Exp = mybir.ActivationFunctionType.Exp

N_CORES = 8
N = 8192
F_IN = 256
D = 64
ALPHA = 0.2

R = N // N_CORES   # rows per core
JC = N // 128      # j-chunks
NG = 8             # mask xbar groups
CG = JC // NG      # chunks per group (8)
HQ = 16            # h-compute groups (4 chunks each)

# masked-multiply split per chunk-PAIR [128, 2048]:
# columns [0:TT_SPLIT] on DVE, rest on GPSIMD
TT_SPLIT = 1216


def build_kernel(ctx: ExitStack, tc: tile.TileContext, mask16_c, x_h, trans_h, transT_h, a12h, outT):
    nc = tc.nc

    singles = ctx.enter_context(tc.tile_pool(name="singles", bufs=1))
    hps_pool = ctx.enter_context(tc.tile_pool(name="hps", bufs=2, space="PSUM"))
    ps_o = ctx.enter_context(tc.tile_pool(name="ps_o", bufs=1, space="PSUM"))
    ps_r = ctx.enter_context(tc.tile_pool(name="ps_r", bufs=1, space="PSUM"))
    work = ctx.enter_context(tc.tile_pool(name="work", bufs=3))
    outp = ctx.enter_context(tc.tile_pool(name="outp", bufs=2))
    dram = ctx.enter_context(tc.tile_pool(name="dram", bufs=1, space="DRAM"))

    # ---- DMA streams.  ALL XBAR transposes share one HWDGE queue (sync):
    # concurrent XBAR streams on two queues corrupt each other.
    # x first (it unblocks all compute), then the mask.
    trans_w = singles.tile([128, 2, D + 2], fp16)
    nc.scalar.dma_start(
        out=trans_w[:, :, 0:D], in_=trans_h.rearrange("(c p) d -> p c d", p=128)
    )
    transT_sb = singles.tile([D, F_IN], fp16)
    nc.scalar.dma_start(out=transT_sb, in_=transT_h[:, :])
    a12_sb = singles.tile([D, 2], fp16)
    nc.scalar.dma_start(out=a12_sb, in_=a12h[:, :])
    # xT[p, fc, j] = x[j, fc*128+p] in one XBAR
    xT = singles.tile([128, 2, N], fp16)
    nc.sync.dma_start(out=xT, in_=x_h[:, :], transpose=True)

    # mask transpose-stream: mt_g[p, k, r] = mask[r, (g*CG+k)*128+p]
    # rotating buffer pool: the XBAR stream self-throttles on consumption
    mtp = ctx.enter_context(tc.tile_pool(name="mtp", bufs=6))
    mts = []
    for g in range(NG):
        mt = mtp.tile([128, CG, R], fp16, tag="mt", name=f"mt{g}")
        cols = slice(g * CG * 128, (g + 1) * CG * 128)
        nc.sync.dma_start(out=mt, in_=mask16_c[:, cols], transpose=True)
        mts.append(mt)

    ident = singles.tile([128, 128], fp16)
    make_identity(nc, ident)
    ones_row_f = singles.tile([1, D], f32)
    nc.vector.memset(ones_row_f, 1.0)

    # ---- w12 = trans.T @ a12  -> moving-operand columns 64:66
    for fc in range(2):
        wps = hps_pool.tile([128, 2], f32, tag="wps", bufs=1)
        nc.tensor.matmul(
            wps, transT_sb[:, fc * 128 : (fc + 1) * 128], a12_sb, start=True, stop=True
        )
        nc.vector.tensor_copy(trans_w[:, fc, D : D + 2], wps)

    # ---- full h/e per 4-chunk groups.  Inputs are rolled per core so the
    # own 1024 rows are always chunks 0..7 (groups 0 and 1).
    h_sb = singles.tile([128, JC, D + 1], fp16)   # [j%128, jc, h | 1]
    nc.vector.memset(h_sb[:, :, D], 1.0)
    c_all = singles.tile([128, JC], fp16)   # exp(.8 e1)
    db_all = singles.tile([128, JC], f32)  # exp(e2)
    b2_all = singles.tile([128, JC], f32)  # exp(.2 e2)
    C_rep = singles.tile([128, R], fp16)

    for q in range(HQ):
        hps = hps_pool.tile([128, 4, D + 2], f32, tag="hps")
        for k in range(4):
            jc = q * 4 + k
            j0 = jc * 128
            for fc in range(2):
                nc.tensor.matmul(
                    hps[:, k, :],
                    xT[:, fc, j0 : j0 + 128],
                    trans_w[:, fc, :],
                    start=(fc == 0),
                    stop=(fc == 1),
                )
        cols = slice(q * 4, q * 4 + 4)
        nc.scalar.copy(h_sb[:, cols, 0:D], hps[:, :, 0:D])
        nc.scalar.activation(c_all[:, cols], hps[:, :, D], Exp, scale=1.0 - ALPHA)
        nc.scalar.activation(db_all[:, cols], hps[:, :, D + 1], Exp, scale=1.0)
        nc.scalar.activation(b2_all[:, cols], hps[:, :, D + 1], Exp, scale=ALPHA)

        if q == 1:
            # ---- C_rep: own-row C values replicated across partitions.
            # Transpose each own column of c_all to a [1, 128] row (PE),
            # assemble [1, R], then broadcast via K=1 matmuls.
            # (No DRAM roundtrip, no DMA in the middle of the XBAR stream.)
            c_row1 = singles.tile([1, R], fp16)
            crps = hps_pool.tile([1, 8, 128], fp16, tag="crps", bufs=1)
            for rb in range(8):
                nc.tensor.transpose(crps[:, rb, :], c_all[:, rb : rb + 1], ident)
            nc.vector.tensor_copy(c_row1, crps.rearrange("p a b -> p (a b)"))
            ones_col = singles.tile([1, 128], fp16)
            nc.vector.memset(ones_col, 1.0)
            for half in range(2):
                crp = hps_pool.tile([128, 512], f32, tag="crp", bufs=1)
                nc.tensor.matmul(
                    crp,
                    ones_col,
                    c_row1[:, half * 512 : (half + 1) * 512],
                    start=True,
                    stop=True,
                )
                nc.vector.tensor_copy(C_rep[:, half * 512 : (half + 1) * 512], crp)

    # ---- main loop: chunk-PAIRS.  Two 4x TS fills, one DVE TT + one
    # GPSIMD TT over the flattened [128, 2048], four accumulated matmuls.
    po = [ps_o.tile([D + 1, 512], f32, tag=f"po{i}", name=f"po{i}") for i in range(2)]
    M = mybir.AluOpType.mult
    NP = JC // 2
    for p in range(NP):
        jc0 = 2 * p
        mt = mts[jc0 // CG]
        k = jc0 % CG
        vp = work.tile([128, 2, R], fp16, tag="vp", bufs=6)
        for half in range(2):
            jc = jc0 + half
            # v = max(C_rep * exp(e2[jc]), exp(.2 e2[jc]))  (= B2 * max(CD, 1))
            nc.vector.tensor_scalar(
                vp[:, half, :],
                C_rep,
                db_all[:, jc : jc + 1],
                b2_all[:, jc : jc + 1],
                M,
                mybir.AluOpType.max,
            )
        # P = mask * v -> pf, split DVE / GPSIMD; GPSIMD piece first
        pf = work.tile([128, 2, R], fp16, tag="pf", bufs=6)
        vf = vp.rearrange("p a b -> p (a b)")
        pff = pf.rearrange("p a b -> p (a b)")
        mf = mt[:, k : k + 2, :].rearrange("p a b -> p (a b)")
        nc.gpsimd.tensor_tensor(pff[:, TT_SPLIT:], vf[:, TT_SPLIT:], mf[:, TT_SPLIT:], M)
        nc.vector.tensor_tensor(pff[:, 0:TT_SPLIT], vf[:, 0:TT_SPLIT], mf[:, 0:TT_SPLIT], M)
        for half in range(2):
            jc = jc0 + half
            for i in range(2):
                nc.tensor.matmul(
                    po[i],
                    h_sb[:, jc, :],
                    pf[:, half, i * 512 : (i + 1) * 512],
                    start=(jc == 0),
                    stop=(jc == JC - 1),
                )

    # ---- normalize: out = numer * (1/denom).  Broadcast the denominator
    # row via a K=1 outer product FIRST, then reciprocal on 64 lanes.
    for i in range(2):
        dcp = outp.tile([1, 512], f32, tag="dcp")
        nc.vector.tensor_copy(dcp, po[i][D : D + 1, :])
        rr = ps_r.tile([D, 512], f32, tag="rr")
        nc.tensor.matmul(rr, ones_row_f, dcp, start=True, stop=True)
        rr_sb = outp.tile([D, 512], f32, tag="rr_sb")
        nc.vector.reciprocal(rr_sb, rr)
        o_t = outp.tile([D, 512], f32, tag="o_t")
        nc.vector.tensor_tensor(o_t, po[i][0:D, :], rr_sb, M)
        nc.gpsimd.dma_start(out=outT[:, i * 512 : (i + 1) * 512], in_=o_t)


def build_nc():
    nc = bacc.Bacc("TRN2", num_devices=N_CORES)
    mask16_c = nc.dram_tensor("mask16_c", [R, N], fp16, kind="ExternalInput")
    x_h = nc.dram_tensor("x_h", [N, F_IN], fp16, kind="ExternalInput")
    trans_h = nc.dram_tensor("trans_h", [F_IN, D], fp16, kind="ExternalInput")
    transT_h = nc.dram_tensor("transT_h", [D, F_IN], fp16, kind="ExternalInput")
    a12h = nc.dram_tensor("a12h", [D, 2], fp16, kind="ExternalInput")
    outT = nc.dram_tensor("outT", [D, R], f32, kind="ExternalOutput")
    with ExitStack() as ctx:
        tc = ctx.enter_context(tile.TileContext(nc))
        build_kernel(
            ctx, tc, mask16_c[:, :], x_h[:, :], trans_h[:, :],
            transT_h[:, :], a12h[:, :], outT[:, :],
        )
    nc.compile()
    return nc


LAST_RESULTS = None


def kernel(x, mask, trans, attn, _trace=False):
    x16 = np.ascontiguousarray(np.asarray(x), dtype=np.float16)
    mask16 = np.ascontiguousarray(np.asarray(mask), dtype=np.float16)
    trans16 = np.ascontiguousarray(np.asarray(trans), dtype=np.float16)
    transT16 = np.ascontiguousarray(np.asarray(trans).T, dtype=np.float16)
    attn = np.asarray(attn, dtype=np.float16)
    a12 = np.ascontiguousarray(np.concatenate([attn[:D], attn[D:]], axis=1))

    nc = build_nc()
    # identical SPMD program on every core: roll x rows / mask columns by
    # -c*R so each core's own rows are always j-chunks 0..7 (a column
    # permutation inside the softmax sum; the result is unchanged)
    in_maps = [
        {
            "mask16_c": np.ascontiguousarray(
                np.roll(mask16[c * R : (c + 1) * R], -c * R, axis=1)
            ),
            "x_h": np.ascontiguousarray(np.roll(x16, -c * R, axis=0)),
            "trans_h": trans16,
            "transT_h": transT16,
            "a12h": a12,
        }
        for c in range(N_CORES)
    ]
    res = run_bass_kernel_spmd(nc, in_maps, list(range(N_CORES)), trace=_trace)
    global LAST_RESULTS
    LAST_RESULTS = res
    out = np.concatenate(
        [res.results[c]["outT"].T for c in range(N_CORES)], axis=0
    )
    return np.ascontiguousarray(out, dtype=np.float32)


if __name__ == "__main__":
    nc = build_nc()
    print("built OK")


# revision 28
# speedup vs baseline: 1.4207x; 1.1190x over previous
"""Trainium2 Bass kernel for nn_NodeAttentionPerMetaPath (GAT-style node attention).

Reference computation (N=8192, F_IN=256, d=64):
    h      = x @ trans                      # [N, d]
    e1     = h @ attn[:d];  e2 = h @ attn[d:]
    scores = leaky_relu(e1 + e2.T, 0.2)     # [N, N]
    masked = where(mask==0, -1e15, scores)
    out    = softmax(masked, axis=1) @ h    # [N, d]

Sharding: rows r across 8 cores (1024 each); every core computes the full
h/e locally from a replicated fp16 x (no collectives at all).

Algebra (exp monotone, alpha<1):
    exp(leaky(e1+e2)) / exp(a*e1) = B2[j] * max(C[r]*D[j], 1)
    C = exp(.8 e1), D = exp(.8 e2), B2 = exp(.2 e2), C*D*B2 = C*exp(e2)
    out = (P @ h) / (P @ 1),  P = mask * B2 * max(CD, 1)
        = mask * max(C_rep * db, b2)        (one fused 4x tensor_scalar)

Layout: scores live TRANSPOSED [j-part, r-free] from the start.  The mask
is packed to fp16 on the host and transpose-loaded straight from DRAM by
the DMA XBAR (dma_start(transpose=True)), so the [N,N] work needs NO PE
transposes and NO PSUM->SBUF copies: per 128-j chunk it is one TS, one
masked multiply (split DVE/GPSIMD), and one accumulated matmul whose
ones-column yields the softmax denominator for free.
"""

from contextlib import ExitStack

import numpy as np

import concourse.bass as bass
import concourse.bacc as bacc
import concourse.mybir as mybir
import concourse.tile as tile
from concourse.bass_utils import run_bass_kernel_spmd
from concourse.masks import make_identity

f32 = mybir.dt.float32
fp16 = mybir.dt.float16

Exp = mybir.ActivationFunctionType.Exp

N_CORES = 8
N = 8192
F_IN = 256
D = 64
ALPHA = 0.2

R = N // N_CORES   # rows per core
JC = N // 128      # j-chunks
NG = 8             # mask xbar groups
CG = JC // NG      # chunks per group (8)
HQ = 16            # h-compute groups (4 chunks each)

# masked-multiply split per chunk-PAIR [128, 2048]:
# columns [0:TT_SPLIT] on DVE, rest on GPSIMD
TT_SPLIT = 1216


def build_kernel(ctx: ExitStack, tc: tile.TileContext, mask16_c, x_h, trans_h, transT_h, a12h, outT):
    nc = tc.nc

    singles = ctx.enter_context(tc.tile_pool(name="singles", bufs=1))
    hps_pool = ctx.enter_context(tc.tile_pool(name="hps", bufs=2, space="PSUM"))
    ps_o = ctx.enter_context(tc.tile_pool(name="ps_o", bufs=1, space="PSUM"))
    ps_r = ctx.enter_context(tc.tile_pool(name="ps_r", bufs=1, space="PSUM"))
    work = ctx.enter_context(tc.tile_pool(name="work", bufs=3))
    outp = ctx.enter_context(tc.tile_pool(name="outp", bufs=1))
    dram = ctx.enter_context(tc.tile_pool(name="dram", bufs=1, space="DRAM"))

    # ---- DMA streams.  ALL XBAR transposes share one HWDGE queue (sync):
    # concurrent XBAR streams on two queues corrupt each other.
    # x first (it unblocks all compute), then the mask.
    trans_w = singles.tile([128, 2, D + 2], fp16)
    nc.scalar.dma_start(
        out=trans_w[:, :, 0:D], in_=trans_h.rearrange("(c p) d -> p c d", p=128)
    )
    transT_sb = singles.tile([D, F_IN], fp16)
    nc.scalar.dma_start(out=transT_sb, in_=transT_h[:, :])
    a12_sb = singles.tile([D, 2], fp16)
    nc.scalar.dma_start(out=a12_sb, in_=a12h[:, :])
    # xT[p, fc, j] = x[j, fc*128+p], four quarter tiles so the h matmuls
    # (and C_rep, which gates the main loop) can start before the whole x
    # transpose lands.  Quarters are interleaved with the first mask groups
    # on the single XBAR queue so the loop's mask data also arrives early.
    xTq = [singles.tile([128, 2, N // 4], fp16, tag=f"xTq{i}", name=f"xTq{i}") for i in range(4)]

    def xbar_x(i):
        nc.sync.dma_start(
            out=xTq[i], in_=x_h[i * (N // 4) : (i + 1) * (N // 4), :], transpose=True
        )

    mtp = ctx.enter_context(tc.tile_pool(name="mtp", bufs=6))
    mts = []

    def xbar_mask(g):
        # mt_g[p, k, r] = mask[r, (g*CG+k)*128+p]; rotating buffer pool:
        # the XBAR stream self-throttles on consumption
        mt = mtp.tile([128, CG, R], fp16, tag="mt", name=f"mt{g}")
        cols = slice(g * CG * 128, (g + 1) * CG * 128)
        nc.sync.dma_start(out=mt, in_=mask16_c[:, cols], transpose=True)
        mts.append(mt)

    xbar_x(0)
    xbar_x(1)
    xbar_mask(0)
    xbar_mask(1)
    xbar_x(2)
    xbar_x(3)
    for g in range(2, NG):
        xbar_mask(g)

    ident = singles.tile([128, 128], fp16)
    make_identity(nc, ident)
    ones_row_f = singles.tile([1, D], f32)
    nc.vector.memset(ones_row_f, 1.0)

    # ---- w12 = trans.T @ a12  -> moving-operand columns 64:66
    for fc in range(2):
        wps = hps_pool.tile([128, 2], f32, tag="wps", bufs=1)
        nc.tensor.matmul(
            wps, transT_sb[:, fc * 128 : (fc + 1) * 128], a12_sb, start=True, stop=True
        )
        nc.vector.tensor_copy(trans_w[:, fc, D : D + 2], wps)

    # ---- full h/e per 4-chunk groups.  Inputs are rolled per core so the
    # own 1024 rows are always chunks 0..7 (groups 0 and 1).
    h_sb = singles.tile([128, JC, D + 1], fp16)   # [j%128, jc, h | 1]
    nc.vector.memset(h_sb[:, :, D], 1.0)
    c_all = singles.tile([128, JC], fp16)   # exp(.8 e1)
    db_all = singles.tile([128, JC], f32)  # exp(e2)
    b2_all = singles.tile([128, JC], f32)  # exp(.2 e2)
    C_rep = singles.tile([128, R], fp16)

    for q in range(HQ):
        hps = hps_pool.tile([128, 4, D + 2], f32, tag="hps")
        for k in range(4):
            jc = q * 4 + k
            xa = xTq[jc // 16]
            j0 = (jc % 16) * 128
            for fc in range(2):
                nc.tensor.matmul(
                    hps[:, k, :],
                    xa[:, fc, j0 : j0 + 128],
                    trans_w[:, fc, :],
                    start=(fc == 0),
                    stop=(fc == 1),
                )
        cols = slice(q * 4, q * 4 + 4)
        nc.scalar.copy(h_sb[:, cols, 0:D], hps[:, :, 0:D])
        nc.scalar.activation(c_all[:, cols], hps[:, :, D], Exp, scale=1.0 - ALPHA)
        nc.scalar.activation(db_all[:, cols], hps[:, :, D + 1], Exp, scale=1.0)
        nc.scalar.activation(b2_all[:, cols], hps[:, :, D + 1], Exp, scale=ALPHA)

        if q == 1:
            # ---- C_rep: own-row C values replicated across partitions.
            # Transpose each own column of c_all to a [1, 128] row (PE),
            # assemble [1, R], then broadcast via K=1 matmuls.
            # (No DRAM roundtrip, no DMA in the middle of the XBAR stream.)
            c_row1 = singles.tile([1, R], fp16)
            crps = hps_pool.tile([1, 8, 128], fp16, tag="crps", bufs=1)
            for rb in range(8):
                nc.tensor.transpose(crps[:, rb, :], c_all[:, rb : rb + 1], ident)
            nc.vector.tensor_copy(c_row1, crps.rearrange("p a b -> p (a b)"))
            ones_col = singles.tile([1, 128], fp16)
            nc.vector.memset(ones_col, 1.0)
            for half in range(2):
                crp = hps_pool.tile([128, 512], f32, tag="crp", bufs=1)
                nc.tensor.matmul(
                    crp,
                    ones_col,
                    c_row1[:, half * 512 : (half + 1) * 512],
                    start=True,
                    stop=True,
                )
                nc.vector.tensor_copy(C_rep[:, half * 512 : (half + 1) * 512], crp)

    # ---- main loop: chunk-PAIRS.  Two 4x TS fills, one DVE TT + one
    # GPSIMD TT over the flattened [128, 2048], four accumulated matmuls.
    po = [ps_o.tile([D + 1, 512], f32, tag=f"po{i}", name=f"po{i}") for i in range(2)]
    M = mybir.AluOpType.mult
    NP = JC // 2
    for p in range(NP):
        jc0 = 2 * p
        mt = mts[jc0 // CG]
        k = jc0 % CG
        vp = work.tile([128, 2, R], fp16, tag="vp", bufs=6)
        for half in range(2):
            jc = jc0 + half
            # v = max(C_rep * exp(e2[jc]), exp(.2 e2[jc]))  (= B2 * max(CD, 1))
            nc.vector.tensor_scalar(
                vp[:, half, :],
                C_rep,
                db_all[:, jc : jc + 1],
                b2_all[:, jc : jc + 1],
                M,
                mybir.AluOpType.max,
            )
        # P = mask * v -> pf, split DVE / GPSIMD; GPSIMD piece first
        pf = work.tile([128, 2, R], fp16, tag="pf", bufs=6)
        vf = vp.rearrange("p a b -> p (a b)")
        pff = pf.rearrange("p a b -> p (a b)")
        mf = mt[:, k : k + 2, :].rearrange("p a b -> p (a b)")
        nc.gpsimd.tensor_tensor(pff[:, TT_SPLIT:], vf[:, TT_SPLIT:], mf[:, TT_SPLIT:], M)
        nc.vector.tensor_tensor(pff[:, 0:TT_SPLIT], vf[:, 0:TT_SPLIT], mf[:, 0:TT_SPLIT], M)
        for half in range(2):
            jc = jc0 + half
            for i in range(2):
                nc.tensor.matmul(
                    po[i],
                    h_sb[:, jc, :],
                    pf[:, half, i * 512 : (i + 1) * 512],
                    start=(jc == 0),
                    stop=(jc == JC - 1),
                )

    # ---- normalize: out = numer * (1/denom).  Broadcast the denominator
    # row via a K=1 outer product FIRST, then reciprocal on 64 lanes.
    # The two halves are interleaved to pipeline across engines.
    dcps, rrs, rsbs, ots = [], [], [], []
    for i in range(2):
        dcp = outp.tile([1, 512], f32, tag=f"dcp{i}", name=f"dcp{i}")
        nc.vector.tensor_copy(dcp, po[i][D : D + 1, :])
        dcps.append(dcp)
    for i in range(2):
        rr = ps_r.tile([D, 512], f32, tag="rr", name=f"rr{i}")
        nc.tensor.matmul(rr, ones_row_f, dcps[i], start=True, stop=True)
        rr_sb = outp.tile([D, 512], f32, tag=f"rr_sb{i}", name=f"rr_sb{i}")
        nc.vector.reciprocal(rr_sb, rr)
        rsbs.append(rr_sb)
    for i in range(2):
        o_t = outp.tile([D, 512], f32, tag=f"o_t{i}", name=f"o_t{i}")
        nc.vector.tensor_tensor(o_t, po[i][0:D, :], rsbs[i], M)
        nc.gpsimd.dma_start(out=outT[:, i * 512 : (i + 1) * 512], in_=o_t)


def build_nc():
    nc = bacc.Bacc("TRN2", num_devices=N_CORES)
    mask16_c = nc.dram_tensor("mask16_c", [R, N], fp16, kind="ExternalInput")
    x_h = nc.dram_tensor("x_h", [N, F_IN], fp16, kind="ExternalInput")
    trans_h = nc.dram_tensor("trans_h", [F_IN, D], fp16, kind="ExternalInput")
    transT_h = nc.dram_tensor("transT_h", [D, F_IN], fp16, kind="ExternalInput")
    a12h = nc.dram_tensor("a12h", [D, 2], fp16, kind="ExternalInput")
    outT = nc.dram_tensor("outT", [D, R], f32, kind="ExternalOutput")
    with ExitStack() as ctx:
        tc = ctx.enter_context(tile.TileContext(nc))
        build_kernel(
            ctx, tc, mask16_c[:, :], x_h[:, :], trans_h[:, :],
            transT_h[:, :], a12h[:, :], outT[:, :],
        )
    nc.compile()
    return nc


LAST_RESULTS = None


def kernel(x, mask, trans, attn, _trace=False):
    x16 = np.ascontiguousarray(np.asarray(x), dtype=np.float16)
    mask16 = np.ascontiguousarray(np.asarray(mask), dtype=np.float16)
    trans16 = np.ascontiguousarray(np.asarray(trans), dtype=np.float16)
    transT16 = np.ascontiguousarray(np.asarray(trans).T, dtype=np.float16)
    attn = np.asarray(attn, dtype=np.float16)
    a12 = np.ascontiguousarray(np.concatenate([attn[:D], attn[D:]], axis=1))

    nc = build_nc()
    # identical SPMD program on every core: roll x rows / mask columns by
    # -c*R so each core's own rows are always j-chunks 0..7 (a column
    # permutation inside the softmax sum; the result is unchanged)
    in_maps = [
        {
            "mask16_c": np.ascontiguousarray(
                np.roll(mask16[c * R : (c + 1) * R], -c * R, axis=1)
            ),
            "x_h": np.ascontiguousarray(np.roll(x16, -c * R, axis=0)),
            "trans_h": trans16,
            "transT_h": transT16,
            "a12h": a12,
        }
        for c in range(N_CORES)
    ]
    res = run_bass_kernel_spmd(nc, in_maps, list(range(N_CORES)), trace=_trace)
    global LAST_RESULTS
    LAST_RESULTS = res
    out = np.concatenate(
        [res.results[c]["outT"].T for c in range(N_CORES)], axis=0
    )
    return np.ascontiguousarray(out, dtype=np.float32)


if __name__ == "__main__":
    nc = build_nc()
    print("built OK")


# revision 29
# speedup vs baseline: 1.4333x; 1.0089x over previous
"""Trainium2 Bass kernel for nn_NodeAttentionPerMetaPath (GAT-style node attention).

Reference computation (N=8192, F_IN=256, d=64):
    h      = x @ trans                      # [N, d]
    e1     = h @ attn[:d];  e2 = h @ attn[d:]
    scores = leaky_relu(e1 + e2.T, 0.2)     # [N, N]
    masked = where(mask==0, -1e15, scores)
    out    = softmax(masked, axis=1) @ h    # [N, d]

Sharding: rows r across 8 cores (1024 each); every core computes the full
h/e locally from a replicated fp16 x (no collectives at all).

Algebra (exp monotone, alpha<1):
    exp(leaky(e1+e2)) / exp(a*e1) = B2[j] * max(C[r]*D[j], 1)
    C = exp(.8 e1), D = exp(.8 e2), B2 = exp(.2 e2), C*D*B2 = C*exp(e2)
    out = (P @ h) / (P @ 1),  P = mask * B2 * max(CD, 1)
        = mask * max(C_rep * db, b2)        (one fused 4x tensor_scalar)

Layout: scores live TRANSPOSED [j-part, r-free] from the start.  The mask
is packed to fp16 on the host and transpose-loaded straight from DRAM by
the DMA XBAR (dma_start(transpose=True)), so the [N,N] work needs NO PE
transposes and NO PSUM->SBUF copies: per 128-j chunk it is one TS, one
masked multiply (split DVE/GPSIMD), and one accumulated matmul whose
ones-column yields the softmax denominator for free.
"""

from contextlib import ExitStack

import numpy as np

import concourse.bass as bass
import concourse.bacc as bacc
import concourse.mybir as mybir
import concourse.tile as tile
from concourse.bass_utils import run_bass_kernel_spmd
from concourse.masks import make_identity

f32 = mybir.dt.float32
fp16 = mybir.dt.float16

Exp = mybir.ActivationFunctionType.Exp

N_CORES = 8
N = 8192
F_IN = 256
D = 64
ALPHA = 0.2

R = N // N_CORES   # rows per core
JC = N // 128      # j-chunks
NG = 8             # mask xbar groups
CG = JC // NG      # chunks per group (8)
HQ = 16            # h-compute groups (4 chunks each)

# masked-multiply split per chunk-PAIR [128, 2048]:
# columns [0:TT_SPLIT] on DVE, rest on GPSIMD
TT_SPLIT = 1216


def build_kernel(ctx: ExitStack, tc: tile.TileContext, mask16_c, x_h, trans_h, transT_h, a12h, outT):
    nc = tc.nc

    singles = ctx.enter_context(tc.tile_pool(name="singles", bufs=1))
    hps_pool = ctx.enter_context(tc.tile_pool(name="hps", bufs=2, space="PSUM"))
    ps_o = ctx.enter_context(tc.tile_pool(name="ps_o", bufs=1, space="PSUM"))
    ps_r = ctx.enter_context(tc.tile_pool(name="ps_r", bufs=1, space="PSUM"))
    work = ctx.enter_context(tc.tile_pool(name="work", bufs=3))
    outp = ctx.enter_context(tc.tile_pool(name="outp", bufs=1))
    dram = ctx.enter_context(tc.tile_pool(name="dram", bufs=1, space="DRAM"))

    # ---- DMA streams.  ALL XBAR transposes share one HWDGE queue (sync):
    # concurrent XBAR streams on two queues corrupt each other.
    # x first (it unblocks all compute), then the mask.
    trans_w = singles.tile([128, 2, D + 2], fp16)
    nc.scalar.dma_start(
        out=trans_w[:, :, 0:D], in_=trans_h.rearrange("(c p) d -> p c d", p=128)
    )
    transT_sb = singles.tile([D, F_IN], fp16)
    nc.scalar.dma_start(out=transT_sb, in_=transT_h[:, :])
    a12_sb = singles.tile([D, 2], fp16)
    nc.scalar.dma_start(out=a12_sb, in_=a12h[:, :])
    # xT[p, fc, j] = x[j, fc*128+p], four quarter tiles so the h matmuls
    # (and C_rep, which gates the main loop) can start before the whole x
    # transpose lands.  Quarters are interleaved with the first mask groups
    # on the single XBAR queue so the loop's mask data also arrives early.
    xTq = [singles.tile([128, 2, N // 4], fp16, tag=f"xTq{i}", name=f"xTq{i}") for i in range(4)]

    def xbar_x(i):
        nc.sync.dma_start(
            out=xTq[i], in_=x_h[i * (N // 4) : (i + 1) * (N // 4), :], transpose=True
        )

    mtp = ctx.enter_context(tc.tile_pool(name="mtp", bufs=6))
    mts = []

    def xbar_mask(g):
        # mt_g[p, k, r] = mask[r, (g*CG+k)*128+p]; rotating buffer pool:
        # the XBAR stream self-throttles on consumption
        mt = mtp.tile([128, CG, R], fp16, tag="mt", name=f"mt{g}")
        cols = slice(g * CG * 128, (g + 1) * CG * 128)
        nc.sync.dma_start(out=mt, in_=mask16_c[:, cols], transpose=True)
        mts.append(mt)

    xbar_x(0)
    xbar_x(1)
    xbar_mask(0)
    xbar_mask(1)
    xbar_x(2)
    xbar_x(3)
    for g in range(2, NG):
        xbar_mask(g)

    ident = singles.tile([128, 128], fp16)
    make_identity(nc, ident)
    ones_row_f = singles.tile([1, D], f32)
    nc.vector.memset(ones_row_f, 1.0)

    # ---- w12 = trans.T @ a12  -> moving-operand columns 64:66
    for fc in range(2):
        wps = hps_pool.tile([128, 2], f32, tag="wps", bufs=1)
        nc.tensor.matmul(
            wps, transT_sb[:, fc * 128 : (fc + 1) * 128], a12_sb, start=True, stop=True
        )
        nc.vector.tensor_copy(trans_w[:, fc, D : D + 2], wps)

    # ---- full h/e per 4-chunk groups.  Inputs are rolled per core so the
    # own 1024 rows are always chunks 0..7 (groups 0 and 1).
    h_sb = singles.tile([128, JC, D + 1], fp16)   # [j%128, jc, h | 1]
    nc.vector.memset(h_sb[:, :, D], 1.0)
    c_all = singles.tile([128, JC], fp16)   # exp(.8 e1)
    db_all = singles.tile([128, JC], f32)  # exp(e2)
    b2_all = singles.tile([128, JC], f32)  # exp(.2 e2)
    C_rep = singles.tile([128, R], fp16)

    for q in range(HQ):
        hps = hps_pool.tile([128, 4, D + 2], f32, tag="hps")
        for k in range(4):
            jc = q * 4 + k
            xa = xTq[jc // 16]
            j0 = (jc % 16) * 128
            for fc in range(2):
                nc.tensor.matmul(
                    hps[:, k, :],
                    xa[:, fc, j0 : j0 + 128],
                    trans_w[:, fc, :],
                    start=(fc == 0),
                    stop=(fc == 1),
                )
        cols = slice(q * 4, q * 4 + 4)
        nc.scalar.copy(h_sb[:, cols, 0:D], hps[:, :, 0:D])
        nc.scalar.activation(c_all[:, cols], hps[:, :, D], Exp, scale=1.0 - ALPHA)
        nc.scalar.activation(db_all[:, cols], hps[:, :, D + 1], Exp, scale=1.0)
        nc.scalar.activation(b2_all[:, cols], hps[:, :, D + 1], Exp, scale=ALPHA)

        if q == 1:
            # ---- C_rep: own-row C values replicated across partitions.
            # Transpose each own column of c_all to a [1, 128] row (PE),
            # assemble [1, R], then broadcast via K=1 matmuls.
            # (No DRAM roundtrip, no DMA in the middle of the XBAR stream.)
            c_row1 = singles.tile([1, R], fp16)
            crps = hps_pool.tile([1, 8, 128], fp16, tag="crps", bufs=1)
            for rb in range(8):
                nc.tensor.transpose(crps[:, rb, :], c_all[:, rb : rb + 1], ident)
            nc.vector.tensor_copy(c_row1, crps.rearrange("p a b -> p (a b)"))
            ones_col = singles.tile([1, 128], fp16)
            nc.vector.memset(ones_col, 1.0)
            for half in range(2):
                crp = hps_pool.tile([128, 512], f32, tag="crp", bufs=1)
                nc.tensor.matmul(
                    crp,
                    ones_col,
                    c_row1[:, half * 512 : (half + 1) * 512],
                    start=True,
                    stop=True,
                )
                nc.vector.tensor_copy(C_rep[:, half * 512 : (half + 1) * 512], crp)

    # ---- main loop: chunk-PAIRS.  Two 4x TS fills, one DVE TT + one
    # GPSIMD TT over the flattened [128, 2048], four accumulated matmuls.
    po = [ps_o.tile([D + 1, 512], f32, tag=f"po{i}", name=f"po{i}") for i in range(2)]
    M = mybir.AluOpType.mult
    # Each pair is processed as four independent 512-column regions, each
    # with exactly ONE producer TT and one consumer matmul, so no matmul
    # ever waits on the other engine's half.  The DVE/GPSIMD split
    # alternates 3/1 and 2/2 regions to balance engine load.
    NP = JC // 2
    for p in range(NP):
        jc0 = 2 * p
        mt = mts[jc0 // CG]
        k = jc0 % CG
        vs = []
        for half in range(2):
            jc = jc0 + half
            v = work.tile([128, R], fp16, tag=f"vc{half}", bufs=6, name=f"v{half}_{p}")
            # v = max(C_rep * exp(e2[jc]), exp(.2 e2[jc]))  (= B2 * max(CD, 1))
            nc.vector.tensor_scalar(
                v,
                C_rep,
                db_all[:, jc : jc + 1],
                b2_all[:, jc : jc + 1],
                M,
                mybir.AluOpType.max,
            )
            vs.append(v)
        n_gp = 1 if p % 2 == 0 else 2
        qs = []
        for reg in range(3, -1, -1):
            half, i = reg // 2, reg % 2
            sl = slice(i * 512, (i + 1) * 512)
            msl = slice(half * R + i * 512, half * R + (i + 1) * 512)
            mf = mt[:, k : k + 2, :].rearrange("p a b -> p (a b)")
            q = work.tile([128, 512], fp16, tag=f"q{reg}", bufs=6, name=f"q{reg}_{p}")
            eng = nc.gpsimd if reg >= 4 - n_gp else nc.vector
            eng.tensor_tensor(q, vs[half][:, sl], mf[:, msl], M)
            qs.append(q)
        qs.reverse()
        for half in range(2):
            jc = jc0 + half
            for i in range(2):
                nc.tensor.matmul(
                    po[i],
                    h_sb[:, jc, :],
                    qs[half * 2 + i],
                    start=(jc == 0),
                    stop=(jc == JC - 1),
                )

    # ---- normalize: out = numer * (1/denom).  Broadcast the denominator
    # row via a K=1 outer product FIRST, then reciprocal on 64 lanes.
    # The two halves are interleaved to pipeline across engines.
    dcps, rrs, rsbs, ots = [], [], [], []
    for i in range(2):
        dcp = outp.tile([1, 512], f32, tag=f"dcp{i}", name=f"dcp{i}")
        nc.vector.tensor_copy(dcp, po[i][D : D + 1, :])
        dcps.append(dcp)
    for i in range(2):
        rr = ps_r.tile([D, 512], f32, tag="rr", name=f"rr{i}")
        nc.tensor.matmul(rr, ones_row_f, dcps[i], start=True, stop=True)
        rr_sb = outp.tile([D, 512], f32, tag=f"rr_sb{i}", name=f"rr_sb{i}")
        nc.vector.reciprocal(rr_sb, rr)
        rsbs.append(rr_sb)
    for i in range(2):
        o_t = outp.tile([D, 512], f32, tag=f"o_t{i}", name=f"o_t{i}")
        nc.vector.tensor_tensor(o_t, po[i][0:D, :], rsbs[i], M)
        nc.gpsimd.dma_start(out=outT[:, i * 512 : (i + 1) * 512], in_=o_t)


def build_nc():
    nc = bacc.Bacc("TRN2", num_devices=N_CORES)
    mask16_c = nc.dram_tensor("mask16_c", [R, N], fp16, kind="ExternalInput")
    x_h = nc.dram_tensor("x_h", [N, F_IN], fp16, kind="ExternalInput")
    trans_h = nc.dram_tensor("trans_h", [F_IN, D], fp16, kind="ExternalInput")
    transT_h = nc.dram_tensor("transT_h", [D, F_IN], fp16, kind="ExternalInput")
    a12h = nc.dram_tensor("a12h", [D, 2], fp16, kind="ExternalInput")
    outT = nc.dram_tensor("outT", [D, R], f32, kind="ExternalOutput")
    with ExitStack() as ctx:
        tc = ctx.enter_context(tile.TileContext(nc))
        build_kernel(
            ctx, tc, mask16_c[:, :], x_h[:, :], trans_h[:, :],
            transT_h[:, :], a12h[:, :], outT[:, :],
        )
    nc.compile()
    return nc


LAST_RESULTS = None


def kernel(x, mask, trans, attn, _trace=False):
    x16 = np.ascontiguousarray(np.asarray(x), dtype=np.float16)
    mask16 = np.ascontiguousarray(np.asarray(mask), dtype=np.float16)
    trans16 = np.ascontiguousarray(np.asarray(trans), dtype=np.float16)
    transT16 = np.ascontiguousarray(np.asarray(trans).T, dtype=np.float16)
    attn = np.asarray(attn, dtype=np.float16)
    a12 = np.ascontiguousarray(np.concatenate([attn[:D], attn[D:]], axis=1))

    nc = build_nc()
    # identical SPMD program on every core: roll x rows / mask columns by
    # -c*R so each core's own rows are always j-chunks 0..7 (a column
    # permutation inside the softmax sum; the result is unchanged)
    in_maps = [
        {
            "mask16_c": np.ascontiguousarray(
                np.roll(mask16[c * R : (c + 1) * R], -c * R, axis=1)
            ),
            "x_h": np.ascontiguousarray(np.roll(x16, -c * R, axis=0)),
            "trans_h": trans16,
            "transT_h": transT16,
            "a12h": a12,
        }
        for c in range(N_CORES)
    ]
    res = run_bass_kernel_spmd(nc, in_maps, list(range(N_CORES)), trace=_trace)
    global LAST_RESULTS
    LAST_RESULTS = res
    out = np.concatenate(
        [res.results[c]["outT"].T for c in range(N_CORES)], axis=0
    )
    return np.ascontiguousarray(out, dtype=np.float32)


if __name__ == "__main__":
    nc = build_nc()
    print("built OK")
